# revision 9
# baseline (speedup 1.0000x reference)
"""Trainium2 Bass kernel for nn_EncoderLayer (pre-norm transformer encoder layer).

Sharding: 8 cores; core c handles batch b=c//2, query rows q0=(c%2)*1024..+1024.
Each core receives its batch's full sequence ROTATED so that its own 1024 query
tokens are rows 0..1023 (a permutation of the keys doesn't change attention).
No collectives: K/V projections are duplicated between the two cores sharing a
batch, everything else is fully parallel.

Datatypes: QKV projections, attn@V and the out-projection run in fp8e4m3 with
DoubleRow perf mode (0.5 cyc/row, two 128-deep k-planes per instruction);
weight matrices carry a x8 scale and V a x16 scale (undone in the psum drains)
to sit in fp8's sweet spot. Scores and the SwiGLU MLP run in bf16. The exp
uses a -1.5 bias (cancels in softmax) so fp8 outputs stay in range, and is
batched two PSUM banks per ScalarE instruction. silu(g)*v is computed as
(1+tanh(g/2))*(g*(v/2)) because tanh shares the ACT table with exp, letting
MLP activations interleave with attention exp without table thrash.

Schedule: two query waves of 512. Wave 0 runs attention for query chunk 0 with
the remaining groups' QKV projections dosed in as PE filler; wave 1 runs
attention for chunk 1 with wave-0's out-projection + LN2 + MLP as filler, so
the ScalarE exp stream hides under PE work; a drain phase finishes wave-1's
MLP. LayerNorm affines are folded into the following projections on the host.
"""
import sys

for p in ("/opt/trn_rl_repo", "/root/.axon_site/_ro/trn_rl_repo"):
    if p not in sys.path:
        sys.path.insert(0, p)

import numpy as np
import ml_dtypes
from contextlib import ExitStack

import concourse.bass as bass
import concourse.mybir as mybir
import concourse.tile as tile
from concourse import bacc
from concourse.masks import make_identity
from concourse.bass_utils import run_bass_kernel_spmd

P = 128
D = 1024
H = 16
QD = 64
S = 2048          # kv tokens per core (full batch sequence)
TQ = 1024         # query tokens per core
INNER = 2730
INNER_PAD = 2816  # 22 * 128
NIT = INNER_PAD // P   # 22 inner tiles
NDT = D // P      # 8 feature tiles
NPR = NDT // 2    # 4 feature-tile pairs (DoubleRow planes)
NT = S // P       # 16 kv token tiles
NTQ = TQ // P     # 8 query token tiles
NG = 4            # head groups (4 heads each)
VW = 80           # per-head V block width (64 vals + ones col + pad to 16B)
EPS = 1e-12
F32 = mybir.dt.float32
BF = mybir.dt.bfloat16
F8 = mybir.dt.float8e4
AF = mybir.ActivationFunctionType
OP = mybir.AluOpType
DRM = mybir.MatmulPerfMode.DoubleRow

E4NP = ml_dtypes.float8_e4m3
BFNP = ml_dtypes.bfloat16


def build_nc():
    nc = bacc.Bacc("TRN2", target_bir_lowering=False, num_devices=8)

    xkv_d = nc.dram_tensor("xkv", [S, D], F32, kind="ExternalInput")
    xq_d = nc.dram_tensor("xq_res", [TQ, D], F32, kind="ExternalInput")
    wq_d = nc.dram_tensor("wq_p", [P, NPR, 2, D], F8, kind="ExternalInput")
    wk_d = nc.dram_tensor("wk_p", [P, NPR, 2, D], F8, kind="ExternalInput")
    wv_d = nc.dram_tensor("wv_p", [P, NPR, 2, D], F8, kind="ExternalInput")
    wo_d = nc.dram_tensor("wo_p", [P, NPR, 2, D], F8, kind="ExternalInput")
    bq_d = nc.dram_tensor("bq_t", [P, NDT], F32, kind="ExternalInput")
    bk_d = nc.dram_tensor("bk_t", [P, NDT], F32, kind="ExternalInput")
    bv_d = nc.dram_tensor("bv_t", [P, H * QD], F32, kind="ExternalInput")
    gw_d = nc.dram_tensor("gw_p", [D, INNER_PAD], BF, kind="ExternalInput")
    vw_d = nc.dram_tensor("vw_p", [D, INNER_PAD], BF, kind="ExternalInput")
    gb_d = nc.dram_tensor("gb_t", [P, NIT], F32, kind="ExternalInput")
    gbh_d = nc.dram_tensor("gbh_t", [P, NIT], F32, kind="ExternalInput")
    vbh_d = nc.dram_tensor("vbh_t", [P, NIT], F32, kind="ExternalInput")
    ow_d = nc.dram_tensor("ow_p", [INNER_PAD, D], BF, kind="ExternalInput")
    out_d = nc.dram_tensor("out", [TQ, D], F32, kind="ExternalOutput")

    with tile.TileContext(nc) as tc, ExitStack() as top:
        misc = top.enter_context(tc.tile_pool(name="misc", bufs=1))

        identity = misc.tile([P, P], BF)
        make_identity(nc, identity)
        eps_t = misc.tile([P, 1], F32)
        nc.gpsimd.memset(eps_t, EPS)
        nb_t = misc.tile([P, 1], F32)
        nc.gpsimd.memset(nb_t, -3.75)
        ones64 = misc.tile([P, QD], F32)
        nc.gpsimd.memset(ones64, 1.0)
        bq_t = misc.tile([P, NDT], F32)
        nc.sync.dma_start(out=bq_t, in_=bq_d[:, :])
        bk_t = misc.tile([P, NDT], F32)
        nc.sync.dma_start(out=bk_t, in_=bk_d[:, :])
        bv_t = misc.tile([P, H * QD], F32)
        nc.sync.dma_start(out=bv_t, in_=bv_d[:, :])
        gb_t = misc.tile([P, NIT], F32)
        nc.sync.dma_start(out=gb_t, in_=gb_d[:, :])
        gbh_t = misc.tile([P, NIT], F32)
        nc.sync.dma_start(out=gbh_t, in_=gbh_d[:, :])
        vbh_t = misc.tile([P, NIT], F32)
        nc.sync.dma_start(out=vbh_t, in_=vbh_d[:, :])

        x2_pool = top.enter_context(tc.tile_pool(name="x2_pool", bufs=1))
        X2 = x2_pool.tile([P, NTQ, D], BF)
        asb_pool = top.enter_context(tc.tile_pool(name="asb_pool", bufs=1))
        attn_sb = asb_pool.tile([P, NDT, TQ], F8)
        wop = top.enter_context(tc.tile_pool(name="wo_pool", bufs=1))
        wo_sb = wop.tile([P, NPR, 2, D], F8)
        nc.sync.dma_start(out=wo_sb, in_=wo_d[:, :, :, :])

        # Batched LayerNorm -> transposed (feature-major) output.
        def ln_stats(scope, src, statp, t):
            with nc.named_scope(scope):
                x_t = src(t)
                stats = statp.tile([P, 2, 6], F32, tag="stats",
                                   name=f"st_{scope}_{t}")
                xv = x_t.rearrange("p (c f) -> p c f", f=512)
                for c in range(2):
                    nc.vector.bn_stats(out=stats[:, c, :], in_=xv[:, c, :])
                mv = statp.tile([P, 2], F32, tag="mv", name=f"mv_{scope}_{t}")
                nc.vector.bn_aggr(out=mv, in_=stats)
                return mv

        def ln_rstd(scope, statp, mv, t):
            with nc.named_scope(scope):
                rstd = statp.tile([P, 1], F32, tag="rstd",
                                  name=f"rstd_{scope}_{t}")
                nc.scalar.activation(out=rstd, in_=mv[:, 1:2], func=AF.Sqrt,
                                     bias=eps_t[:, 0:1], scale=1.0)
                nc.vector.reciprocal(out=rstd, in_=rstd)
                return rstd

        def ln_norm_t(scope, src, nrmp, tpp, mv, rstd, dst4, dst_dt, t):
            with nc.named_scope(scope):
                nrm = nrmp.tile([P, D], BF, tag="nrm", name=f"n_{scope}_{t}")
                nc.vector.tensor_scalar(
                    out=nrm, in0=src(t), scalar1=mv[:, 0:1], scalar2=rstd,
                    op0=OP.subtract, op1=OP.mult)
                for half in range(2):
                    tp = tpp.tile([P, 512], BF, tag="fa",
                                  name=f"tp_{scope}_{t}_{half}")
                    for j in range(4):
                        dt = half * 4 + j
                        nc.tensor.transpose(
                            tp[:, j * P:(j + 1) * P],
                            nrm[:, dt * P:(dt + 1) * P], identity)
                    nc.scalar.activation(
                        out=dst4(half, t),
                        in_=tp.rearrange("p (j f) -> p j f", f=P),
                        func=AF.Copy)
                _ = dst_dt  # dst dtype is carried by dst4's target tile

        # ---------------- attention-wide pools ---------------------------
        with tc.tile_pool(name="qkv_sb", bufs=1) as qkvp, \
             tc.tile_pool(name="expp", bufs=3) as expp, \
             tc.tile_pool(name="rvp", bufs=3) as rvp, \
             tc.tile_pool(name="sc_ps", bufs=2, space="PSUM") as scps, \
             tc.tile_pool(name="us_ps", bufs=2, space="PSUM") as usps, \
             tc.tile_pool(name="fil_ps", bufs=2, space="PSUM") as filps:

            Q_all = qkvp.tile([P, NG, 2, TQ], F8)
            K_all = qkvp.tile([P, NG, 2, S], F8)
            V_all = qkvp.tile([P, NG, NT, 4, VW], F8)

            def qkv_mms(g, wpool):
                """Closures emitting group g's QKV projections (fp8 DR)."""
                mms = []
                st = {}

                def alloc():
                    with nc.named_scope(f"qkv{g}"):
                        for nm, wd in (("wq", wq_d), ("wk", wk_d),
                                       ("wv", wv_d)):
                            wt = wpool.tile([P, NPR, 2, 256], F8, tag=nm,
                                            name=f"{nm}{g}")
                            nc.sync.dma_start(
                                out=wt,
                                in_=wd[:, :, :, g * 256:(g + 1) * 256])
                            st[nm] = wt
                        nc.vector.tensor_copy(
                            out=V_all[:, g, :, :, QD],
                            in_=ones64.rearrange("p (k h) -> p k h", h=4)
                            [:, 0:NT, :])
                mms.append(alloc)

                cell = {}

                def mk_qk(which, pj, chunk, pr):
                    def f():
                        with nc.named_scope(f"qkv{g}"):
                            if pr == 0:
                                cell[which, pj, chunk] = filps.tile(
                                    [P, 512], F32, tag="fa",
                                    name=f"{which}ps{g}{pj}{chunk}")
                            ps = cell[which, pj, chunk]
                            w = st[which][:, pr, :, pj * P:(pj + 1) * P]
                            dst = Q_all if which == "wq" else K_all
                            nc.tensor.matmul(
                                ps, w,
                                hT[:, 2 * pr:2 * pr + 2,
                                   chunk * 512:(chunk + 1) * 512],
                                start=(pr == 0), stop=(pr == NPR - 1),
                                perf_mode=DRM)
                            if pr == NPR - 1:
                                b = bq_t if which == "wq" else bk_t
                                dt_g = g * 2 + pj
                                nc.vector.tensor_scalar(
                                    out=dst[:, g, pj,
                                            chunk * 512:(chunk + 1) * 512],
                                    in0=ps, scalar1=0.125,
                                    scalar2=b[:, dt_g:dt_g + 1],
                                    op0=OP.mult, op1=OP.add)
                    return f

                def mk_v(kt2, pr):
                    def f():
                        with nc.named_scope(f"qkv{g}"):
                            if pr == 0:
                                cell["v", kt2] = filps.tile(
                                    [P, 512], F32, tag="fa",
                                    name=f"vps{g}_{kt2}")
                            ps = cell["v", kt2]
                            nc.tensor.matmul(
                                ps[:, 0:256],
                                hT[:, 2 * pr:2 * pr + 2,
                                   kt2 * P:(kt2 + 1) * P],
                                st["wv"][:, pr, :, :],
                                start=(pr == 0), stop=(pr == NPR - 1),
                                perf_mode=DRM)
                            if pr == NPR - 1:
                                nc.vector.scalar_tensor_tensor(
                                    out=V_all[:, g, kt2, :, 0:QD],
                                    in0=ps[:, 0:256].rearrange(
                                        "p (h c) -> p h c", c=QD),
                                    scalar=2.0,
                                    in1=bv_t.rearrange("p (h c) -> p h c", c=QD)
                                    [:, 4 * g:4 * g + 4, :],
                                    op0=OP.mult, op1=OP.add)
                    return f

                for pj in range(2):
                    for qc in range(2):
                        for pr in range(NPR):
                            mms.append(mk_qk("wq", pj, qc, pr))
                    for c in range(4):
                        for pr in range(NPR):
                            mms.append(mk_qk("wk", pj, c, pr))
                for kt2 in range(NT):
                    for pr in range(NPR):
                        mms.append(mk_v(kt2, pr))
                return mms

            def attn_wave(qc, group_fill):
                """Attention for query chunk qc as ONE software pipeline over
                all 64 (g, pj, kb) steps (no flush at group boundaries, so
                ScalarE's exp stream never drains); group_fill[g] closures
                are dosed in during group g's 16 steps and flushed right
                before group g+1's first scores."""
                steps = [(g, pj, kb) for g in range(NG)
                         for pj in range(2) for kb in range(8)]
                fi = [0] * NG
                acc = [0.0] * NG

                def fill(g, frac):
                    filler = group_fill[g]
                    acc[g] += frac
                    while acc[g] >= 1.0 and fi[g] < len(filler):
                        filler[fi[g]]()
                        fi[g] += 1
                        acc[g] -= 1.0

                def flush(g):
                    acc[g] += len(group_fill[g])
                    fill(g, 0.0)

                exps = {}
                us = {}

                def emit_scores(step):
                    g, pj, kb = step
                    with nc.named_scope(f"attn{g}w{qc}"):
                        pA = scps.tile([P, 2, 512], F32, tag="s", name="pA")
                        pB = scps.tile([P, 2, 512], F32, tag="s", name="pB")
                        for i, kc in enumerate((2 * kb, 2 * kb + 1)):
                            nc.tensor.matmul(
                                pA[:, i, :],
                                K_all[0:64, g, pj, kc * P:(kc + 1) * P],
                                Q_all[0:64, g, pj,
                                      qc * 512:(qc + 1) * 512],
                                start=True, stop=True,
                                tile_position=(0, 0))
                            nc.tensor.matmul(
                                pB[:, i, :],
                                K_all[64:128, g, pj, kc * P:(kc + 1) * P],
                                Q_all[64:128, g, pj,
                                      qc * 512:(qc + 1) * 512],
                                start=True, stop=True,
                                tile_position=(64, 0))
                        eA = expp.tile([P, 2, 512], F8, tag="eA", name="eA")
                        nc.scalar.activation(out=eA, in_=pA, func=AF.Exp,
                                             bias=nb_t[:, 0:1], scale=0.125)
                        eB = expp.tile([P, 2, 512], F8, tag="eB", name="eB")
                        nc.scalar.activation(out=eB, in_=pB, func=AF.Exp,
                                             bias=nb_t[:, 0:1], scale=0.125)
                        exps[step] = (eA, eB)

                def emit_attnv(step):
                    g, pj, kb = step
                    with nc.named_scope(f"attn{g}w{qc}"):
                        if kb == 0:
                            us[g, pj] = [
                                usps.tile([65, 512], F32, tag="u",
                                          name=f"uA{g}{pj}{qc}"),
                                usps.tile([65, 512], F32, tag="u",
                                          name=f"uB{g}{pj}{qc}")]
                        eA, eB = exps.pop(step)
                        for side, e in ((0, eA), (1, eB)):
                            hh = pj * 2 + side
                            nc.tensor.matmul(
                                us[g, pj][side],
                                V_all[:, g, 2 * kb:2 * kb + 2, hh, 0:65],
                                e, start=(kb == 0), stop=(kb == 7),
                                perf_mode=DRM)

                def emit_norm(g, pj):
                    dt_g = g * 2 + pj
                    with nc.named_scope(f"attn{g}w{qc}"):
                        for side in (0, 1):
                            u = us[g, pj][side]
                            rv = rvp.tile([1, 512], F32, tag="rv", name="rv")
                            nc.vector.reciprocal(out=rv[0:1, :],
                                                 in_=u[64:65, :])
                            bc = rvp.tile([64, 512], F32, tag="bc", name="bc")
                            nc.gpsimd.partition_broadcast(bc, rv[0:1, :])
                            nc.vector.tensor_tensor(
                                out=attn_sb[side * 64:(side + 1) * 64,
                                            dt_g,
                                            qc * 512:(qc + 1) * 512],
                                in0=u[0:64, :], in1=bc, op=OP.mult)
                        del us[g, pj]

                LEAD = 1
                for i in range(len(steps) + LEAD):
                    if i < len(steps):
                        g = steps[i][0]
                        if i % 16 == 0 and i > 0:
                            flush(g - 1)  # fillers due before group g starts
                        emit_scores(steps[i])
                    if i >= LEAD:
                        step = steps[i - LEAD]
                        emit_attnv(step)
                        if step[2] == 7:
                            emit_norm(step[0], step[1])
                    if i < len(steps):
                        fill(steps[i][0], len(group_fill[steps[i][0]]) / 16.0)
                for g in range(NG):
                    flush(g)

            # ------------- phase A: LN1 + QKV + wave 0 --------------------
            with tc.tile_pool(name="hT_pool", bufs=1) as hT_pool:
                hT = hT_pool.tile([P, NDT, S], F8)

                with tc.tile_pool(name="ln1x", bufs=4) as xp, \
                     tc.tile_pool(name="ln1n", bufs=3) as nrmp, \
                     tc.tile_pool(name="ln1s", bufs=3) as statp:
                    for t in range(NT):
                        x_t = xp.tile([P, D], F32, tag="x", name=f"x_ln1_{t}")
                        nc.sync.dma_start(out=x_t,
                                          in_=xkv_d[t * P:(t + 1) * P, :])
                        src = lambda _t, _x=x_t: _x
                        mv = ln_stats("ln1", src, statp, t)
                        rstd = ln_rstd("ln1", statp, mv, t)
                        ln_norm_t("ln1", src, nrmp, filps, mv, rstd,
                                  lambda half, _t: hT[:, half * 4:half * 4 + 4,
                                                      _t * P:(_t + 1) * P],
                                  F8, t)

                with tc.tile_pool(name="wtl", bufs=2) as wpool:
                    for q in qkv_mms(0, wpool):
                        q()
                    attn_wave(0, [qkv_mms(1, wpool), qkv_mms(2, wpool),
                                  qkv_mms(3, wpool), []])

            # ------------- phase B: wave 1 + MLP --------------------------
            with tc.tile_pool(name="h2T_pool", bufs=1) as h2T_pool, \
                 tc.tile_pool(name="m_pool", bufs=1) as mp, \
                 tc.tile_pool(name="ln2s", bufs=4) as statp2, \
                 tc.tile_pool(name="ln2n", bufs=3) as nrmp2, \
                 tc.tile_pool(name="opx", bufs=3) as oxp, \
                 tc.tile_pool(name="gvw", bufs=3) as gvwp, \
                 tc.tile_pool(name="gvt", bufs=3) as gvtp, \
                 tc.tile_pool(name="oww", bufs=6) as owwp, \
                 tc.tile_pool(name="owd", bufs=3) as owdp:
                h2T = h2T_pool.tile([P, NDT, TQ], BF)
                m_sb = mp.tile([P, NIT, 512], BF)  # one wave at a time

                def outproj_cls(mt):
                    cls = []
                    st = {}

                    def load():
                        with nc.named_scope("outproj"):
                            st["xq"] = oxp.tile([P, D], F32, tag="xq",
                                                name=f"xq{mt}")
                            nc.sync.dma_start(
                                out=st["xq"],
                                in_=xq_d[mt * P:(mt + 1) * P, :])
                    cls.append(load)

                    def mk(ncx, pr):
                        def f():
                            with nc.named_scope("outproj"):
                                if pr == 0:
                                    st[ncx] = filps.tile(
                                        [P, 512], F32, tag="fa",
                                        name=f"ops{mt}{ncx}")
                                nc.tensor.matmul(
                                    st[ncx],
                                    attn_sb[:, 2 * pr:2 * pr + 2,
                                            mt * P:(mt + 1) * P],
                                    wo_sb[:, pr, :,
                                          ncx * 512:(ncx + 1) * 512],
                                    start=(pr == 0), stop=(pr == NPR - 1),
                                    perf_mode=DRM)
                                if pr == NPR - 1:
                                    nc.vector.scalar_tensor_tensor(
                                        out=X2[:, mt,
                                               ncx * 512:(ncx + 1) * 512],
                                        in0=st[ncx], scalar=1.0 / 128.0,
                                        in1=st["xq"][:,
                                                     ncx * 512:(ncx + 1) * 512],
                                        op0=OP.mult, op1=OP.add)
                        return f
                    for ncx in range(2):
                        for pr in range(NPR):
                            cls.append(mk(ncx, pr))
                    return cls

                def ln2_cls(mts):
                    """LN2 for the given query tiles; sqrt batched so the
                    ACT-table swaps away from Exp happen once."""
                    cls = []
                    mvs = {}
                    rstds = {}

                    def mk_stats(t):
                        def f():
                            mvs[t] = ln_stats("ln2", lambda _t: X2[:, _t, :],
                                              statp2, t)
                        return f

                    def rstd_batch():
                        for t in mts:
                            rstds[t] = ln_rstd("ln2", statp2, mvs[t], t)

                    def mk_fin(t):
                        def f():
                            ln_norm_t("ln2", lambda _t: X2[:, _t, :], nrmp2,
                                      filps, mvs[t], rstds[t],
                                      lambda half, _t: h2T[:, half * 4:half * 4 + 4,
                                                           _t * P:(_t + 1) * P],
                                      BF, t)
                        return f
                    for t in mts:
                        cls.append(mk_stats(t))
                    cls.append(rstd_batch)
                    for t in mts:
                        cls.append(mk_fin(t))
                    return cls

                def gv_cls(it, qc2, use_sc_psum=False):
                    cls = []
                    st = {}

                    def load():
                        with nc.named_scope("mlp_gv"):
                            st["g"] = gvwp.tile([P, NDT, P], BF, tag="gsl",
                                                name=f"gsl{it}")
                            nc.sync.dma_start(
                                out=st["g"], in_=gw_d[:, it * P:(it + 1) * P]
                                .rearrange("(kt p) n -> p kt n", p=P))
                            st["v"] = gvwp.tile([P, NDT, P], BF, tag="vsl",
                                                name=f"vsl{it}")
                            nc.sync.dma_start(
                                out=st["v"], in_=vw_d[:, it * P:(it + 1) * P]
                                .rearrange("(kt p) n -> p kt n", p=P))
                    cls.append(load)

                    def alloc_ps():
                        if use_sc_psum:
                            a = scps.tile([P, 2, 512], F32, tag="s",
                                          name=f"gvps{it}")
                            st["gps"], st["vps"] = a[:, 0, :], a[:, 1, :]
                        else:
                            st["gps"] = filps.tile([P, 512], F32, tag="fa",
                                                   name=f"psg{it}")
                            st["vps"] = filps.tile([P, 512], F32, tag="fa",
                                                   name=f"psv{it}")

                    def mk_mm(which, kt):
                        def f():
                            with nc.named_scope("mlp_gv"):
                                if which == "g" and kt == 0:
                                    alloc_ps()
                                nc.tensor.matmul(
                                    st[which + "ps"], st[which][:, kt, :],
                                    h2T[:, kt, qc2 * 512:(qc2 + 1) * 512],
                                    start=(kt == 0), stop=(kt == NDT - 1))
                        return f
                    for kt in range(NDT):
                        cls.append(mk_mm("g", kt))
                    for kt in range(NDT):
                        cls.append(mk_mm("v", kt))

                    def drain():
                        with nc.named_scope("mlp_gv"):
                            g_sb = gvtp.tile([P, 512], BF, tag="g_sb",
                                             name=f"g_sb{it}")
                            nc.vector.tensor_copy(out=g_sb, in_=st["gps"])
                            vh = gvtp.tile([P, 512], BF, tag="vh",
                                           name=f"vh{it}")
                            nc.vector.tensor_scalar_add(
                                out=vh, in0=st["vps"],
                                scalar1=vbh_t[:, it:it + 1])
                            t_t = gvtp.tile([P, 512], BF, tag="t",
                                            name=f"t{it}")
                            nc.scalar.activation(out=t_t, in_=g_sb,
                                                 func=AF.Tanh,
                                                 bias=gbh_t[:, it:it + 1],
                                                 scale=0.5)
                            gvh = gvtp.tile([P, 512], BF, tag="gvh",
                                            name=f"gvh{it}")
                            nc.vector.scalar_tensor_tensor(
                                out=gvh, in0=g_sb,
                                scalar=gb_t[:, it:it + 1], in1=vh,
                                op0=OP.add, op1=OP.mult)
                            nc.vector.scalar_tensor_tensor(
                                out=m_sb[:, it, :], in0=t_t, scalar=1.0,
                                in1=gvh, op0=OP.add, op1=OP.mult)
                    cls.append(drain)
                    return cls

                def ow_pass_cls(mt, qc2, accs):
                    """One ow output tile (128 rows x 1024): 2 psum chains
                    over all 22 inner tiles. accs() -> (tile0, tile1)."""
                    cls = []
                    st = {}

                    def mk_it(it):
                        def f():
                            with nc.named_scope("mlp_ow"):
                                if it == 0:
                                    st["a"] = accs()
                                owt = owwp.tile([P, D], BF, tag="owt",
                                                name=f"owt{mt}_{it}")
                                nc.sync.dma_start(
                                    out=owt,
                                    in_=ow_d[it * P:(it + 1) * P, :])
                                mloc = mt - qc2 * 4
                                for ncx in range(2):
                                    nc.tensor.matmul(
                                        st["a"][ncx],
                                        m_sb[:, it, mloc * P:(mloc + 1) * P],
                                        owt[:, ncx * 512:(ncx + 1) * 512],
                                        start=(it == 0), stop=(it == NIT - 1))
                        return f
                    for it in range(NIT):
                        cls.append(mk_it(it))

                    def drain():
                        with nc.named_scope("mlp_ow"):
                            for ncx in range(2):
                                ot = owdp.tile([P, 512], F32, tag="ot",
                                               name=f"ot{mt}{ncx}")
                                nc.vector.tensor_tensor(
                                    out=ot, in0=st["a"][ncx],
                                    in1=X2[:, mt, ncx * 512:(ncx + 1) * 512],
                                    op=OP.add)
                                nc.sync.dma_start(
                                    out=out_d[mt * P:(mt + 1) * P,
                                              ncx * 512:(ncx + 1) * 512],
                                    in_=ot)
                    cls.append(drain)
                    return cls

                def fil_accs():
                    a0 = filps.tile([P, 512], F32, tag="fa", name="owa0")
                    a1 = filps.tile([P, 512], F32, tag="fa", name="owa1")
                    return (a0, a1)

                def sc_accs():
                    a = scps.tile([P, 2, 512], F32, tag="s", name="owa2")
                    return (a[:, 0, :], a[:, 1, :])

                # wave-1 filler: wave-0's outproj, LN2, MLP
                w1_fill = []
                for mt in range(4):
                    w1_fill += outproj_cls(mt)
                w1_fill += ln2_cls(range(4))
                for it in range(NIT):
                    w1_fill += gv_cls(it, 0)
                for mt in range(2):
                    w1_fill += ow_pass_cls(mt, 0, fil_accs)

                nseg = (len(w1_fill) + NG - 1) // NG
                attn_wave(1, [w1_fill[i * nseg:(i + 1) * nseg]
                              for i in range(NG)])

                # drain: rest of wave-0 ow, then wave-1 MLP
                for mt in range(2, 4):
                    for c in ow_pass_cls(mt, 0, fil_accs):
                        c()
                for mt in range(4, 8):
                    for c in outproj_cls(mt):
                        c()
                for c in ln2_cls(range(4, 8)):
                    c()
                for it in range(NIT):
                    for c in gv_cls(it, 1, use_sc_psum=True):
                        c()
                for mt in range(4, 8):
                    accs = sc_accs if mt % 2 == 0 else fil_accs
                    for c in ow_pass_cls(mt, 1, accs):
                        c()
    return nc


def make_core_inputs(X, src_padding_mask, n1_w, n1_b, n2_w, n2_b,
                     wq, bq, wk, bk, wv, bv, wo, bo,
                     gw, gb, vw, vb, ow, ob):
    """Build the per-core device input dicts from full numpy inputs.
    LayerNorm affines are folded into the consuming projections:
    h = z*w + b  =>  h @ W + c = z @ (diag(w) W) + (b W + c)."""
    X = np.asarray(X, np.float32)
    f = lambda a: np.ascontiguousarray(np.asarray(a, np.float32))
    n1_w, n1_b = f(n1_w), f(n1_b)
    n2_w, n2_b = f(n2_w), f(n2_b)
    wq_f = n1_w[:, None] * f(wq)
    wk_f = n1_w[:, None] * f(wk)
    wv_f = n1_w[:, None] * f(wv)
    bq_f = f(bq) + n1_b @ f(wq)
    bk_f = f(bk) + n1_b @ f(wk)
    bv_f = f(bv) + n1_b @ f(wv)
    gw_f = n2_w[:, None] * f(gw)
    vw_f = n2_w[:, None] * f(vw)
    gb_f = f(gb) + n2_b @ f(gw)
    vb_f = f(vb) + n2_b @ f(vw)

    # pack [D, D] -> [P, NPR, 2, D] fp8 with x8 scale (DoubleRow k-planes)
    def pack8(w):
        w8 = (8.0 * w).reshape(NPR, 2, P, D).transpose(2, 0, 1, 3)
        return np.ascontiguousarray(w8).astype(E4NP)

    col = lambda v: f(v).reshape(NDT, P).T.copy()       # [P, 8] per-partition
    coli = lambda v: np.pad(f(v), (0, INNER_PAD - INNER)).reshape(NIT, P).T.copy()
    shared = {
        "wq_p": pack8(wq_f), "wk_p": pack8(wk_f), "wv_p": pack8(wv_f),
        "wo_p": pack8(f(wo)),
        "bq_t": col(bq_f), "bk_t": col(bk_f),
        "bv_t": np.tile(16.0 * bv_f, (P, 1)).astype(np.float32),
        "gw_p": np.pad(gw_f, ((0, 0), (0, INNER_PAD - INNER))).astype(BFNP),
        "vw_p": np.pad(0.5 * vw_f,
                       ((0, 0), (0, INNER_PAD - INNER))).astype(BFNP),
        "gb_t": coli(gb_f), "gbh_t": coli(0.5 * gb_f),
        "vbh_t": coli(0.5 * vb_f),
        "ow_p": np.pad(f(ow), ((0, INNER_PAD - INNER), (0, 0))).astype(BFNP),
    }
    res_b = (f(bo) + f(ob))[None, :]
    in_maps = []
    for c in range(8):
        b, q0 = c // 2, (c % 2) * TQ
        xroll = np.ascontiguousarray(
            np.concatenate([X[b, q0:], X[b, :q0]], axis=0))
        m = dict(shared)
        m["xkv"] = xroll
        m["xq_res"] = np.ascontiguousarray(xroll[:TQ] + res_b)
        in_maps.append(m)
    return in_maps


_CACHE = {}


def _get_compiled():
    if "nc" not in _CACHE:
        nc = build_nc()
        nc.compile()
        _CACHE["nc"] = nc
    return _CACHE["nc"]


def kernel(**inputs) -> np.ndarray:
    nc = _get_compiled()
    in_maps = make_core_inputs(**inputs)
    res = run_bass_kernel_spmd(nc, in_maps, core_ids=list(range(8)))
    B_full, S_full = 4, 2048
    out = np.empty((B_full, S_full, D), np.float32)
    for c in range(8):
        b, q0 = c // 2, (c % 2) * TQ
        out[b, q0:q0 + TQ, :] = res.results[c]["out"]
    return out


# revision 13
# speedup vs baseline: 1.1183x; 1.1183x over previous
"""Trainium2 Bass kernel for nn_EncoderLayer (pre-norm transformer encoder layer).

Sharding: 8 cores; core c handles batch b=c//2, query rows q0=(c%2)*1024..+1024.
Each core receives its batch's full sequence ROTATED so that its own 1024 query
tokens are rows 0..1023 (a permutation of the keys doesn't change attention).
No collectives: K/V projections are duplicated between the two cores sharing a
batch, everything else is fully parallel.

Datatypes: QKV projections, attn@V and the out-projection run in fp8e4m3 with
DoubleRow perf mode (0.5 cyc/row, two 128-deep k-planes per instruction);
weight matrices carry a x8 scale and V a x16 scale (undone in the psum drains)
to sit in fp8's sweet spot. Scores and the SwiGLU MLP run in bf16. The exp
uses a -1.5 bias (cancels in softmax) so fp8 outputs stay in range, and is
batched two PSUM banks per ScalarE instruction. silu(g)*v is computed as
(1+tanh(g/2))*(g*(v/2)) because tanh shares the ACT table with exp, letting
MLP activations interleave with attention exp without table thrash.

Schedule: two query waves of 512. Wave 0 runs attention for query chunk 0 with
the remaining groups' QKV projections dosed in as PE filler; wave 1 runs
attention for chunk 1 with wave-0's out-projection + LN2 + MLP as filler, so
the ScalarE exp stream hides under PE work; a drain phase finishes wave-1's
MLP. LayerNorm affines are folded into the following projections on the host.
"""
import sys

for p in ("/opt/trn_rl_repo", "/root/.axon_site/_ro/trn_rl_repo"):
    if p not in sys.path:
        sys.path.insert(0, p)

import numpy as np
import ml_dtypes
from contextlib import ExitStack

import concourse.bass as bass
import concourse.mybir as mybir
import concourse.tile as tile
from concourse import bacc
from concourse.masks import make_identity
from concourse.bass_utils import run_bass_kernel_spmd

P = 128
D = 1024
H = 16
QD = 64
S = 2048          # kv tokens per core (full batch sequence)
TQ = 1024         # query tokens per core
INNER = 2730
INNER_PAD = 2816  # 22 * 128
NIT = INNER_PAD // P   # 22 inner tiles
NDT = D // P      # 8 feature tiles
NPR = NDT // 2    # 4 feature-tile pairs (DoubleRow planes)
NT = S // P       # 16 kv token tiles
NTQ = TQ // P     # 8 query token tiles
NG = 4            # head groups (4 heads each)
VW = 80           # per-head V block width (64 vals + ones col + pad to 16B)
EPS = 1e-12
F32 = mybir.dt.float32
BF = mybir.dt.bfloat16
F8 = mybir.dt.float8e4
AF = mybir.ActivationFunctionType
OP = mybir.AluOpType
DRM = mybir.MatmulPerfMode.DoubleRow

E4NP = ml_dtypes.float8_e4m3
BFNP = ml_dtypes.bfloat16


def build_nc():
    nc = bacc.Bacc("TRN2", target_bir_lowering=False, num_devices=8)

    xkv_d = nc.dram_tensor("xkv", [S, D], F32, kind="ExternalInput")
    xq_d = nc.dram_tensor("xq_res", [TQ, D], F32, kind="ExternalInput")
    wq_d = nc.dram_tensor("wq_p", [P, NPR, 2, D], F8, kind="ExternalInput")
    wk_d = nc.dram_tensor("wk_p", [P, NPR, 2, D], F8, kind="ExternalInput")
    wv_d = nc.dram_tensor("wv_p", [P, NPR, 2, D], F8, kind="ExternalInput")
    wo_d = nc.dram_tensor("wo_p", [P, NPR, 2, D], F8, kind="ExternalInput")
    bq_d = nc.dram_tensor("bq_t", [P, NDT], F32, kind="ExternalInput")
    bk_d = nc.dram_tensor("bk_t", [P, NDT], F32, kind="ExternalInput")
    bv_d = nc.dram_tensor("bv_t", [P, H * QD], F32, kind="ExternalInput")
    gw_d = nc.dram_tensor("gw_p", [D, INNER_PAD], BF, kind="ExternalInput")
    vw_d = nc.dram_tensor("vw_p", [D, INNER_PAD], BF, kind="ExternalInput")
    gb_d = nc.dram_tensor("gb_t", [P, NIT], F32, kind="ExternalInput")
    gbh_d = nc.dram_tensor("gbh_t", [P, NIT], F32, kind="ExternalInput")
    vbh_d = nc.dram_tensor("vbh_t", [P, NIT], F32, kind="ExternalInput")
    ow_d = nc.dram_tensor("ow_p", [INNER_PAD, D], BF, kind="ExternalInput")
    out_d = nc.dram_tensor("out", [TQ, D], F32, kind="ExternalOutput")

    with tile.TileContext(nc) as tc, ExitStack() as top:
        misc = top.enter_context(tc.tile_pool(name="misc", bufs=1))

        identity = misc.tile([P, P], BF)
        make_identity(nc, identity)
        eps_t = misc.tile([P, 1], F32)
        nc.gpsimd.memset(eps_t, EPS)
        nb_t = misc.tile([P, 1], F32)
        nc.gpsimd.memset(nb_t, -3.75)
        ones64 = misc.tile([P, QD], F32)
        nc.gpsimd.memset(ones64, 1.0)
        bq_t = misc.tile([P, NDT], F32)
        nc.sync.dma_start(out=bq_t, in_=bq_d[:, :])
        bk_t = misc.tile([P, NDT], F32)
        nc.sync.dma_start(out=bk_t, in_=bk_d[:, :])
        bv_t = misc.tile([P, H * QD], F32)
        nc.sync.dma_start(out=bv_t, in_=bv_d[:, :])
        gb_t = misc.tile([P, NIT], F32)
        nc.sync.dma_start(out=gb_t, in_=gb_d[:, :])
        gbh_t = misc.tile([P, NIT], F32)
        nc.sync.dma_start(out=gbh_t, in_=gbh_d[:, :])
        vbh_t = misc.tile([P, NIT], F32)
        nc.sync.dma_start(out=vbh_t, in_=vbh_d[:, :])

        x2_pool = top.enter_context(tc.tile_pool(name="x2_pool", bufs=1))
        X2 = x2_pool.tile([P, NTQ, D], BF)
        asb_pool = top.enter_context(tc.tile_pool(name="asb_pool", bufs=1))
        attn_sb = asb_pool.tile([P, NDT, TQ], F8)
        wop = top.enter_context(tc.tile_pool(name="wo_pool", bufs=1))
        wo_sb = wop.tile([P, NPR, 2, D], F8)
        nc.sync.dma_start(out=wo_sb, in_=wo_d[:, :, :, :])

        # Batched LayerNorm -> transposed (feature-major) output.
        def ln_stats(scope, src, statp, t):
            with nc.named_scope(scope):
                x_t = src(t)
                stats = statp.tile([P, 2, 6], F32, tag="stats",
                                   name=f"st_{scope}_{t}")
                xv = x_t.rearrange("p (c f) -> p c f", f=512)
                for c in range(2):
                    nc.vector.bn_stats(out=stats[:, c, :], in_=xv[:, c, :])
                mv = statp.tile([P, 2], F32, tag="mv", name=f"mv_{scope}_{t}")
                nc.vector.bn_aggr(out=mv, in_=stats)
                return mv

        def ln_rstd(scope, statp, mv, t):
            with nc.named_scope(scope):
                rstd = statp.tile([P, 1], F32, tag="rstd",
                                  name=f"rstd_{scope}_{t}")
                nc.scalar.activation(out=rstd, in_=mv[:, 1:2], func=AF.Sqrt,
                                     bias=eps_t[:, 0:1], scale=1.0)
                nc.vector.reciprocal(out=rstd, in_=rstd)
                return rstd

        def ln_norm_t(scope, src, nrmp, tpp, mv, rstd, dst4, dst_dt, t):
            with nc.named_scope(scope):
                nrm = nrmp.tile([P, D], BF, tag="nrm", name=f"n_{scope}_{t}")
                nc.vector.tensor_scalar(
                    out=nrm, in0=src(t), scalar1=mv[:, 0:1], scalar2=rstd,
                    op0=OP.subtract, op1=OP.mult)
                for half in range(2):
                    tp = tpp.tile([P, 512], BF, tag="fa",
                                  name=f"tp_{scope}_{t}_{half}")
                    for j in range(4):
                        dt = half * 4 + j
                        nc.tensor.transpose(
                            tp[:, j * P:(j + 1) * P],
                            nrm[:, dt * P:(dt + 1) * P], identity)
                    nc.scalar.activation(
                        out=dst4(half, t),
                        in_=tp.rearrange("p (j f) -> p j f", f=P),
                        func=AF.Copy)
                _ = dst_dt  # dst dtype is carried by dst4's target tile

        # ---------------- attention-wide pools ---------------------------
        with tc.tile_pool(name="qkv_sb", bufs=1) as qkvp, \
             tc.tile_pool(name="expp", bufs=3) as expp, \
             tc.tile_pool(name="rvp", bufs=3) as rvp, \
             tc.tile_pool(name="sc_ps", bufs=2, space="PSUM") as scps, \
             tc.tile_pool(name="us_ps", bufs=2, space="PSUM") as usps, \
             tc.tile_pool(name="fil_ps", bufs=2, space="PSUM") as filps:

            Q_all = qkvp.tile([P, NG, 2, TQ], F8)
            K_all = qkvp.tile([P, NG, 2, S], F8)
            V_all = qkvp.tile([P, NG, NT, 4, VW], F8)

            def qkv_mms(g, wpool):
                """Closures emitting group g's QKV projections (fp8 DR)."""
                mms = []
                st = {}

                def alloc():
                    with nc.named_scope(f"qkv{g}"):
                        for nm, wd in (("wq", wq_d), ("wk", wk_d),
                                       ("wv", wv_d)):
                            wt = wpool.tile([P, NPR, 2, 256], F8, tag=nm,
                                            name=f"{nm}{g}")
                            nc.sync.dma_start(
                                out=wt,
                                in_=wd[:, :, :, g * 256:(g + 1) * 256])
                            st[nm] = wt
                        nc.vector.tensor_copy(
                            out=V_all[:, g, :, :, QD],
                            in_=ones64.rearrange("p (k h) -> p k h", h=4)
                            [:, 0:NT, :])
                mms.append(alloc)

                cell = {}

                def mk_qk(which, pj, chunk, pr):
                    def f():
                        with nc.named_scope(f"qkv{g}"):
                            if pr == 0:
                                cell[which, pj, chunk] = filps.tile(
                                    [P, 512], F32, tag="fa",
                                    name=f"{which}ps{g}{pj}{chunk}")
                            ps = cell[which, pj, chunk]
                            w = st[which][:, pr, :, pj * P:(pj + 1) * P]
                            dst = Q_all if which == "wq" else K_all
                            nc.tensor.matmul(
                                ps, w,
                                hT[:, 2 * pr:2 * pr + 2,
                                   chunk * 512:(chunk + 1) * 512],
                                start=(pr == 0), stop=(pr == NPR - 1),
                                perf_mode=DRM)
                            if pr == NPR - 1:
                                b = bq_t if which == "wq" else bk_t
                                dt_g = g * 2 + pj
                                nc.vector.tensor_scalar(
                                    out=dst[:, g, pj,
                                            chunk * 512:(chunk + 1) * 512],
                                    in0=ps, scalar1=0.125,
                                    scalar2=b[:, dt_g:dt_g + 1],
                                    op0=OP.mult, op1=OP.add)
                    return f

                def mk_v(kt2, pr):
                    def f():
                        with nc.named_scope(f"qkv{g}"):
                            if pr == 0:
                                cell["v", kt2] = filps.tile(
                                    [P, 512], F32, tag="fa",
                                    name=f"vps{g}_{kt2}")
                            ps = cell["v", kt2]
                            nc.tensor.matmul(
                                ps[:, 0:256],
                                hT[:, 2 * pr:2 * pr + 2,
                                   kt2 * P:(kt2 + 1) * P],
                                st["wv"][:, pr, :, :],
                                start=(pr == 0), stop=(pr == NPR - 1),
                                perf_mode=DRM)
                            if pr == NPR - 1:
                                nc.vector.scalar_tensor_tensor(
                                    out=V_all[:, g, kt2, :, 0:QD],
                                    in0=ps[:, 0:256].rearrange(
                                        "p (h c) -> p h c", c=QD),
                                    scalar=2.0,
                                    in1=bv_t.rearrange("p (h c) -> p h c", c=QD)
                                    [:, 4 * g:4 * g + 4, :],
                                    op0=OP.mult, op1=OP.add)
                    return f

                for pj in range(2):
                    for qc in range(2):
                        for pr in range(NPR):
                            mms.append(mk_qk("wq", pj, qc, pr))
                    for c in range(4):
                        for pr in range(NPR):
                            mms.append(mk_qk("wk", pj, c, pr))
                for kt2 in range(NT):
                    for pr in range(NPR):
                        mms.append(mk_v(kt2, pr))
                return mms

            def attn_wave(qc, group_fill):
                """Attention for query chunk qc as ONE software pipeline over
                all 64 (g, pj, kb) steps (no flush at group boundaries, so
                ScalarE's exp stream never drains); group_fill[g] closures
                are dosed in during group g's 16 steps and flushed right
                before group g+1's first scores."""
                steps = [(g, pj, kb) for g in range(NG)
                         for pj in range(2) for kb in range(8)]
                fi = [0] * NG
                acc = [0.0] * NG

                def fill(g, frac):
                    filler = group_fill[g]
                    acc[g] += frac
                    while acc[g] >= 1.0 and fi[g] < len(filler):
                        filler[fi[g]]()
                        fi[g] += 1
                        acc[g] -= 1.0

                def flush(g):
                    acc[g] += len(group_fill[g])
                    fill(g, 0.0)

                exps = {}
                us = {}

                def emit_scores(step):
                    g, pj, kb = step
                    with nc.named_scope(f"attn{g}w{qc}"):
                        pA = scps.tile([P, 2, 512], F32, tag="s", name="pA")
                        pB = scps.tile([P, 2, 512], F32, tag="s", name="pB")
                        for i, kc in enumerate((2 * kb, 2 * kb + 1)):
                            nc.tensor.matmul(
                                pA[:, i, :],
                                K_all[0:64, g, pj, kc * P:(kc + 1) * P],
                                Q_all[0:64, g, pj,
                                      qc * 512:(qc + 1) * 512],
                                start=True, stop=True,
                                tile_position=(0, 0))
                            nc.tensor.matmul(
                                pB[:, i, :],
                                K_all[64:128, g, pj, kc * P:(kc + 1) * P],
                                Q_all[64:128, g, pj,
                                      qc * 512:(qc + 1) * 512],
                                start=True, stop=True,
                                tile_position=(64, 0))
                        eA = expp.tile([P, 2, 512], F8, tag="eA", name="eA")
                        nc.scalar.activation(out=eA, in_=pA, func=AF.Exp,
                                             bias=nb_t[:, 0:1], scale=0.125)
                        eB = expp.tile([P, 2, 512], F8, tag="eB", name="eB")
                        nc.scalar.activation(out=eB, in_=pB, func=AF.Exp,
                                             bias=nb_t[:, 0:1], scale=0.125)
                        exps[step] = (eA, eB)

                def emit_attnv(step):
                    g, pj, kb = step
                    with nc.named_scope(f"attn{g}w{qc}"):
                        if kb == 0:
                            us[g, pj] = [
                                usps.tile([65, 512], F32, tag="u",
                                          name=f"uA{g}{pj}{qc}"),
                                usps.tile([65, 512], F32, tag="u",
                                          name=f"uB{g}{pj}{qc}")]
                        eA, eB = exps.pop(step)
                        for side, e in ((0, eA), (1, eB)):
                            hh = pj * 2 + side
                            nc.tensor.matmul(
                                us[g, pj][side],
                                V_all[:, g, 2 * kb:2 * kb + 2, hh, 0:65],
                                e, start=(kb == 0), stop=(kb == 7),
                                perf_mode=DRM)

                def emit_norm(g, pj):
                    dt_g = g * 2 + pj
                    with nc.named_scope(f"attn{g}w{qc}"):
                        for side in (0, 1):
                            u = us[g, pj][side]
                            # evacuate PSUM fast so the next chain's
                            # accumulator slot frees without waiting for the
                            # reciprocal chain
                            ucp = rvp.tile([65, 512], F32, tag="ucp",
                                           name="ucp")
                            nc.vector.tensor_copy(out=ucp, in_=u)
                            rv = rvp.tile([1, 512], F32, tag="rv", name="rv")
                            nc.vector.reciprocal(out=rv[0:1, :],
                                                 in_=ucp[64:65, :])
                            bc = rvp.tile([64, 512], F32, tag="bc", name="bc")
                            nc.gpsimd.partition_broadcast(bc, rv[0:1, :])
                            nc.vector.tensor_tensor(
                                out=attn_sb[side * 64:(side + 1) * 64,
                                            dt_g,
                                            qc * 512:(qc + 1) * 512],
                                in0=ucp[0:64, :], in1=bc, op=OP.mult)
                        del us[g, pj]

                LEAD = 1
                for i in range(len(steps) + LEAD):
                    if i < len(steps):
                        g = steps[i][0]
                        if i % 16 == 0 and i > 0:
                            flush(g - 1)  # fillers due before group g starts
                        emit_scores(steps[i])
                    if i >= LEAD:
                        step = steps[i - LEAD]
                        emit_attnv(step)
                        if step[2] == 7:
                            emit_norm(step[0], step[1])
                    if i < len(steps):
                        fill(steps[i][0], len(group_fill[steps[i][0]]) / 16.0)
                for g in range(NG):
                    flush(g)

            # ------------- phase A: LN1 + QKV + wave 0 --------------------
            with tc.tile_pool(name="hT_pool", bufs=1) as hT_pool:
                hT = hT_pool.tile([P, NDT, S], F8)

                with tc.tile_pool(name="ln1x", bufs=4) as xp, \
                     tc.tile_pool(name="ln1n", bufs=3) as nrmp, \
                     tc.tile_pool(name="ln1s", bufs=3) as statp:
                    for t in range(NT):
                        x_t = xp.tile([P, D], F32, tag="x", name=f"x_ln1_{t}")
                        nc.sync.dma_start(out=x_t,
                                          in_=xkv_d[t * P:(t + 1) * P, :])
                        src = lambda _t, _x=x_t: _x
                        mv = ln_stats("ln1", src, statp, t)
                        rstd = ln_rstd("ln1", statp, mv, t)
                        ln_norm_t("ln1", src, nrmp, filps, mv, rstd,
                                  lambda half, _t: hT[:, half * 4:half * 4 + 4,
                                                      _t * P:(_t + 1) * P],
                                  F8, t)

                with tc.tile_pool(name="wtl", bufs=2) as wpool:
                    for q in qkv_mms(0, wpool):
                        q()
                    attn_wave(0, [qkv_mms(1, wpool), qkv_mms(2, wpool),
                                  qkv_mms(3, wpool), []])

            # ------------- phase B: wave 1 + MLP --------------------------
            with tc.tile_pool(name="h2T_pool", bufs=1) as h2T_pool, \
                 tc.tile_pool(name="m_pool", bufs=1) as mp, \
                 tc.tile_pool(name="ln2s", bufs=4) as statp2, \
                 tc.tile_pool(name="ln2n", bufs=3) as nrmp2, \
                 tc.tile_pool(name="opx", bufs=3) as oxp, \
                 tc.tile_pool(name="gvw", bufs=3) as gvwp, \
                 tc.tile_pool(name="gvt", bufs=3) as gvtp, \
                 tc.tile_pool(name="oww", bufs=6) as owwp, \
                 tc.tile_pool(name="owd", bufs=3) as owdp:
                h2T = h2T_pool.tile([P, NDT, TQ], BF)
                m_sb = mp.tile([P, NIT, 512], BF)  # one wave at a time

                def outproj_cls(mt):
                    cls = []
                    st = {}

                    def load():
                        with nc.named_scope("outproj"):
                            st["xq"] = oxp.tile([P, D], F32, tag="xq",
                                                name=f"xq{mt}")
                            nc.sync.dma_start(
                                out=st["xq"],
                                in_=xq_d[mt * P:(mt + 1) * P, :])
                    cls.append(load)

                    def mk(ncx, pr):
                        def f():
                            with nc.named_scope("outproj"):
                                if pr == 0:
                                    st[ncx] = filps.tile(
                                        [P, 512], F32, tag="fa",
                                        name=f"ops{mt}{ncx}")
                                nc.tensor.matmul(
                                    st[ncx],
                                    attn_sb[:, 2 * pr:2 * pr + 2,
                                            mt * P:(mt + 1) * P],
                                    wo_sb[:, pr, :,
                                          ncx * 512:(ncx + 1) * 512],
                                    start=(pr == 0), stop=(pr == NPR - 1),
                                    perf_mode=DRM)
                                if pr == NPR - 1:
                                    nc.vector.scalar_tensor_tensor(
                                        out=X2[:, mt,
                                               ncx * 512:(ncx + 1) * 512],
                                        in0=st[ncx], scalar=1.0 / 128.0,
                                        in1=st["xq"][:,
                                                     ncx * 512:(ncx + 1) * 512],
                                        op0=OP.mult, op1=OP.add)
                        return f
                    for ncx in range(2):
                        for pr in range(NPR):
                            cls.append(mk(ncx, pr))
                    return cls

                def ln2_cls(mts):
                    """LN2 for the given query tiles; sqrt batched so the
                    ACT-table swaps away from Exp happen once."""
                    cls = []
                    mvs = {}
                    rstds = {}

                    def mk_stats(t):
                        def f():
                            mvs[t] = ln_stats("ln2", lambda _t: X2[:, _t, :],
                                              statp2, t)
                        return f

                    def rstd_batch():
                        for t in mts:
                            rstds[t] = ln_rstd("ln2", statp2, mvs[t], t)

                    def mk_fin(t):
                        def f():
                            ln_norm_t("ln2", lambda _t: X2[:, _t, :], nrmp2,
                                      filps, mvs[t], rstds[t],
                                      lambda half, _t: h2T[:, half * 4:half * 4 + 4,
                                                           _t * P:(_t + 1) * P],
                                      BF, t)
                        return f
                    for t in mts:
                        cls.append(mk_stats(t))
                    cls.append(rstd_batch)
                    for t in mts:
                        cls.append(mk_fin(t))
                    return cls

                def gv_cls(it, qc2, use_sc_psum=False):
                    cls = []
                    st = {}

                    def load():
                        with nc.named_scope("mlp_gv"):
                            st["g"] = gvwp.tile([P, NDT, P], BF, tag="gsl",
                                                name=f"gsl{it}")
                            nc.sync.dma_start(
                                out=st["g"], in_=gw_d[:, it * P:(it + 1) * P]
                                .rearrange("(kt p) n -> p kt n", p=P))
                            st["v"] = gvwp.tile([P, NDT, P], BF, tag="vsl",
                                                name=f"vsl{it}")
                            nc.sync.dma_start(
                                out=st["v"], in_=vw_d[:, it * P:(it + 1) * P]
                                .rearrange("(kt p) n -> p kt n", p=P))
                    cls.append(load)

                    def alloc_ps():
                        if use_sc_psum:
                            a = scps.tile([P, 2, 512], F32, tag="s",
                                          name=f"gvps{it}")
                            st["gps"], st["vps"] = a[:, 0, :], a[:, 1, :]
                        else:
                            st["gps"] = filps.tile([P, 512], F32, tag="fa",
                                                   name=f"psg{it}")
                            st["vps"] = filps.tile([P, 512], F32, tag="fa",
                                                   name=f"psv{it}")

                    def mk_mm(which, kt):
                        def f():
                            with nc.named_scope("mlp_gv"):
                                if which == "g" and kt == 0:
                                    alloc_ps()
                                nc.tensor.matmul(
                                    st[which + "ps"], st[which][:, kt, :],
                                    h2T[:, kt, qc2 * 512:(qc2 + 1) * 512],
                                    start=(kt == 0), stop=(kt == NDT - 1))
                        return f
                    for kt in range(NDT):
                        cls.append(mk_mm("g", kt))
                    for kt in range(NDT):
                        cls.append(mk_mm("v", kt))

                    def drain():
                        with nc.named_scope("mlp_gv"):
                            g_sb = gvtp.tile([P, 512], BF, tag="g_sb",
                                             name=f"g_sb{it}")
                            nc.vector.tensor_copy(out=g_sb, in_=st["gps"])
                            vh = gvtp.tile([P, 512], BF, tag="vh",
                                           name=f"vh{it}")
                            nc.vector.tensor_scalar_add(
                                out=vh, in0=st["vps"],
                                scalar1=vbh_t[:, it:it + 1])
                            t_t = gvtp.tile([P, 512], BF, tag="t",
                                            name=f"t{it}")
                            nc.scalar.activation(out=t_t, in_=g_sb,
                                                 func=AF.Tanh,
                                                 bias=gbh_t[:, it:it + 1],
                                                 scale=0.5)
                            gvh = gvtp.tile([P, 512], BF, tag="gvh",
                                            name=f"gvh{it}")
                            nc.vector.scalar_tensor_tensor(
                                out=gvh, in0=g_sb,
                                scalar=gb_t[:, it:it + 1], in1=vh,
                                op0=OP.add, op1=OP.mult)
                            nc.vector.scalar_tensor_tensor(
                                out=m_sb[:, it, :], in0=t_t, scalar=1.0,
                                in1=gvh, op0=OP.add, op1=OP.mult)
                    cls.append(drain)
                    return cls

                def ow_pass_cls(mts, qc2, accs_fn):
                    """ow output tiles for the given mts (2 psum chains each)
                    sharing one streamed pass over the 22 weight tiles.
                    accs_fn() -> list of 2*len(mts) accumulator APs."""
                    cls = []
                    st = {}

                    def mk_it(it):
                        def f():
                            with nc.named_scope("mlp_ow"):
                                if it == 0:
                                    st["a"] = accs_fn()
                                owt = owwp.tile([P, D], BF, tag="owt",
                                                name=f"owt{mts[0]}_{it}")
                                nc.sync.dma_start(
                                    out=owt,
                                    in_=ow_d[it * P:(it + 1) * P, :])
                                for j, mt in enumerate(mts):
                                    mloc = mt - qc2 * 4
                                    for ncx in range(2):
                                        nc.tensor.matmul(
                                            st["a"][2 * j + ncx],
                                            m_sb[:, it,
                                                 mloc * P:(mloc + 1) * P],
                                            owt[:, ncx * 512:(ncx + 1) * 512],
                                            start=(it == 0),
                                            stop=(it == NIT - 1))
                        return f
                    for it in range(NIT):
                        cls.append(mk_it(it))

                    def drain():
                        with nc.named_scope("mlp_ow"):
                            for j, mt in enumerate(mts):
                                for ncx in range(2):
                                    ot = owdp.tile([P, 512], F32, tag="ot",
                                                   name=f"ot{mt}{ncx}")
                                    nc.vector.tensor_tensor(
                                        out=ot, in0=st["a"][2 * j + ncx],
                                        in1=X2[:, mt,
                                               ncx * 512:(ncx + 1) * 512],
                                        op=OP.add)
                                    nc.sync.dma_start(
                                        out=out_d[mt * P:(mt + 1) * P,
                                                  ncx * 512:(ncx + 1) * 512],
                                        in_=ot)
                    cls.append(drain)
                    return cls

                def fil_accs():
                    a0 = filps.tile([P, 512], F32, tag="fa", name="owa0")
                    a1 = filps.tile([P, 512], F32, tag="fa", name="owa1")
                    return [a0, a1]

                def quad_accs():
                    a = scps.tile([P, 2, 512], F32, tag="s", name="owa2")
                    b0 = filps.tile([P, 512], F32, tag="fa", name="owb0")
                    b1 = filps.tile([P, 512], F32, tag="fa", name="owb1")
                    return [a[:, 0, :], a[:, 1, :], b0, b1]

                # wave-1 filler: wave-0's outproj, LN2, MLP
                w1_fill = []
                for mt in range(4):
                    w1_fill += outproj_cls(mt)
                w1_fill += ln2_cls(range(4))
                for it in range(NIT):
                    w1_fill += gv_cls(it, 0)
                for mt in range(2):
                    w1_fill += ow_pass_cls([mt], 0, fil_accs)

                nseg = (len(w1_fill) + NG - 1) // NG
                attn_wave(1, [w1_fill[i * nseg:(i + 1) * nseg]
                              for i in range(NG)])

                def sc4_accs():
                    a = scps.tile([P, 2, 512], F32, tag="s", name="owa4")
                    b = scps.tile([P, 2, 512], F32, tag="s", name="owb4")
                    return [a[:, 0, :], a[:, 1, :], b[:, 0, :], b[:, 1, :]]

                # drain: merge the DVE/Scalar-led wave-1 outproj+LN2 stream
                # with the PE-led remaining wave-0 ow pass so neither engine
                # idles; psum: ow uses "s" slots, outproj/LN2 use "fa".
                da = []
                for mt in range(4, 8):
                    da += outproj_cls(mt)
                da += ln2_cls(range(4, 8))
                db = ow_pass_cls([2, 3], 0, sc4_accs)
                k = max(len(da), len(db))
                for i in range(k):
                    if i < len(da):
                        da[i]()
                    if i < len(db):
                        db[i]()
                for it in range(NIT):
                    for c in gv_cls(it, 1, use_sc_psum=True):
                        c()
                for mts in ([4, 5], [6, 7]):
                    for c in ow_pass_cls(mts, 1, quad_accs):
                        c()
    return nc


def make_core_inputs(X, src_padding_mask, n1_w, n1_b, n2_w, n2_b,
                     wq, bq, wk, bk, wv, bv, wo, bo,
                     gw, gb, vw, vb, ow, ob):
    """Build the per-core device input dicts from full numpy inputs.
    LayerNorm affines are folded into the consuming projections:
    h = z*w + b  =>  h @ W + c = z @ (diag(w) W) + (b W + c)."""
    X = np.asarray(X, np.float32)
    f = lambda a: np.ascontiguousarray(np.asarray(a, np.float32))
    n1_w, n1_b = f(n1_w), f(n1_b)
    n2_w, n2_b = f(n2_w), f(n2_b)
    wq_f = n1_w[:, None] * f(wq)
    wk_f = n1_w[:, None] * f(wk)
    wv_f = n1_w[:, None] * f(wv)
    bq_f = f(bq) + n1_b @ f(wq)
    bk_f = f(bk) + n1_b @ f(wk)
    bv_f = f(bv) + n1_b @ f(wv)
    gw_f = n2_w[:, None] * f(gw)
    vw_f = n2_w[:, None] * f(vw)
    gb_f = f(gb) + n2_b @ f(gw)
    vb_f = f(vb) + n2_b @ f(vw)

    # pack [D, D] -> [P, NPR, 2, D] fp8 with x8 scale (DoubleRow k-planes)
    def pack8(w):
        w8 = (8.0 * w).reshape(NPR, 2, P, D).transpose(2, 0, 1, 3)
        return np.ascontiguousarray(w8).astype(E4NP)

    col = lambda v: f(v).reshape(NDT, P).T.copy()       # [P, 8] per-partition
    coli = lambda v: np.pad(f(v), (0, INNER_PAD - INNER)).reshape(NIT, P).T.copy()
    shared = {
        "wq_p": pack8(wq_f), "wk_p": pack8(wk_f), "wv_p": pack8(wv_f),
        "wo_p": pack8(f(wo)),
        "bq_t": col(bq_f), "bk_t": col(bk_f),
        "bv_t": np.tile(16.0 * bv_f, (P, 1)).astype(np.float32),
        "gw_p": np.pad(gw_f, ((0, 0), (0, INNER_PAD - INNER))).astype(BFNP),
        "vw_p": np.pad(0.5 * vw_f,
                       ((0, 0), (0, INNER_PAD - INNER))).astype(BFNP),
        "gb_t": coli(gb_f), "gbh_t": coli(0.5 * gb_f),
        "vbh_t": coli(0.5 * vb_f),
        "ow_p": np.pad(f(ow), ((0, INNER_PAD - INNER), (0, 0))).astype(BFNP),
    }
    res_b = (f(bo) + f(ob))[None, :]
    in_maps = []
    for c in range(8):
        b, q0 = c // 2, (c % 2) * TQ
        xroll = np.ascontiguousarray(
            np.concatenate([X[b, q0:], X[b, :q0]], axis=0))
        m = dict(shared)
        m["xkv"] = xroll
        m["xq_res"] = np.ascontiguousarray(xroll[:TQ] + res_b)
        in_maps.append(m)
    return in_maps


_CACHE = {}


def _get_compiled():
    if "nc" not in _CACHE:
        nc = build_nc()
        nc.compile()
        _CACHE["nc"] = nc
    return _CACHE["nc"]


def kernel(**inputs) -> np.ndarray:
    nc = _get_compiled()
    in_maps = make_core_inputs(**inputs)
    res = run_bass_kernel_spmd(nc, in_maps, core_ids=list(range(8)))
    B_full, S_full = 4, 2048
    out = np.empty((B_full, S_full, D), np.float32)
    for c in range(8):
        b, q0 = c // 2, (c % 2) * TQ
        out[b, q0:q0 + TQ, :] = res.results[c]["out"]
    return out


# revision 24
# speedup vs baseline: 1.1875x; 1.0619x over previous
"""Trainium2 Bass kernel for nn_EncoderLayer (pre-norm transformer encoder layer).

Sharding: 8 cores; core c handles batch b=c//2, query rows q0=(c%2)*1024..+1024.
Each core receives its batch's full sequence ROTATED so that its own 1024 query
tokens are rows 0..1023 (a permutation of the keys doesn't change attention).
No collectives: K/V projections are duplicated between the two cores sharing a
batch, everything else is fully parallel.

Datatypes: QKV projections, attn@V and the out-projection run in fp8e4m3 with
DoubleRow perf mode (0.5 cyc/row, two 128-deep k-planes per instruction);
weight matrices carry a x8 scale and V a x16 scale (undone in the psum drains)
to sit in fp8's sweet spot. Scores and the SwiGLU MLP run in bf16. The exp
uses a -1.5 bias (cancels in softmax) so fp8 outputs stay in range, and is
batched two PSUM banks per ScalarE instruction. silu(g)*v is computed as
(1+tanh(g/2))*(g*(v/2)) because tanh shares the ACT table with exp, letting
MLP activations interleave with attention exp without table thrash.

Schedule: two query waves of 512. Wave 0 runs attention for query chunk 0 with
the remaining groups' QKV projections dosed in as PE filler; wave 1 runs
attention for chunk 1 with wave-0's out-projection + LN2 + MLP as filler, so
the ScalarE exp stream hides under PE work; a drain phase finishes wave-1's
MLP. LayerNorm affines are folded into the following projections on the host.
"""
import sys

for p in ("/opt/trn_rl_repo", "/root/.axon_site/_ro/trn_rl_repo"):
    if p not in sys.path:
        sys.path.insert(0, p)

import numpy as np
import ml_dtypes
from contextlib import ExitStack

import concourse.bass as bass
import concourse.mybir as mybir
import concourse.tile as tile
from concourse import bacc
from concourse.masks import make_identity
from concourse.bass_utils import run_bass_kernel_spmd

P = 128
D = 1024
H = 16
QD = 64
S = 2048          # kv tokens per core (full batch sequence)
TQ = 1024         # query tokens per core
INNER = 2730
INNER_PAD = 2816  # 22 * 128
NIT = INNER_PAD // P   # 22 inner tiles
NDT = D // P      # 8 feature tiles
NPR = NDT // 2    # 4 feature-tile pairs (DoubleRow planes)
NT = S // P       # 16 kv token tiles
NTQ = TQ // P     # 8 query token tiles
NG = 4            # head groups (4 heads each)
VW = 80           # per-head V block width (64 vals + ones col + pad to 16B)
EPS = 1e-12
F32 = mybir.dt.float32
BF = mybir.dt.bfloat16
F8 = mybir.dt.float8e4
AF = mybir.ActivationFunctionType
OP = mybir.AluOpType
DRM = mybir.MatmulPerfMode.DoubleRow

E4NP = ml_dtypes.float8_e4m3
BFNP = ml_dtypes.bfloat16


def build_nc():
    nc = bacc.Bacc("TRN2", target_bir_lowering=False, num_devices=8)

    xkv_d = nc.dram_tensor("xkv", [S, D], F32, kind="ExternalInput")
    xq_d = nc.dram_tensor("xq_res", [TQ, D], F32, kind="ExternalInput")
    wq_d = nc.dram_tensor("wq_p", [P, NPR, 2, D], F8, kind="ExternalInput")
    wk_d = nc.dram_tensor("wk_p", [P, NPR, 2, D], F8, kind="ExternalInput")
    wv_d = nc.dram_tensor("wv_p", [P, NPR, 2, D], F8, kind="ExternalInput")
    wo_d = nc.dram_tensor("wo_p", [P, NPR, 2, D], F8, kind="ExternalInput")
    bq_d = nc.dram_tensor("bq_t", [P, NDT], F32, kind="ExternalInput")
    bk_d = nc.dram_tensor("bk_t", [P, NDT], F32, kind="ExternalInput")
    bv_d = nc.dram_tensor("bv_t", [P, H * QD], F32, kind="ExternalInput")
    gw_d = nc.dram_tensor("gw_p", [D, INNER_PAD], BF, kind="ExternalInput")
    vw_d = nc.dram_tensor("vw_p", [D, INNER_PAD], BF, kind="ExternalInput")
    gb_d = nc.dram_tensor("gb_t", [P, NIT], F32, kind="ExternalInput")
    gbh_d = nc.dram_tensor("gbh_t", [P, NIT], F32, kind="ExternalInput")
    vbh_d = nc.dram_tensor("vbh_t", [P, NIT], F32, kind="ExternalInput")
    ow_d = nc.dram_tensor("ow_p", [INNER_PAD, D], BF, kind="ExternalInput")
    out_d = nc.dram_tensor("out", [TQ, D], F32, kind="ExternalOutput")

    with tile.TileContext(nc) as tc, ExitStack() as top:
        misc = top.enter_context(tc.tile_pool(name="misc", bufs=1))

        identity = misc.tile([P, P], BF)
        make_identity(nc, identity)
        eps_t = misc.tile([P, 1], F32)
        nc.gpsimd.memset(eps_t, EPS)
        nb_t = misc.tile([P, 1], F32)
        nc.gpsimd.memset(nb_t, -3.75)
        ones64 = misc.tile([P, QD], F32)
        nc.gpsimd.memset(ones64, 1.0)
        bq_t = misc.tile([P, NDT], F32)
        nc.sync.dma_start(out=bq_t, in_=bq_d[:, :])
        bk_t = misc.tile([P, NDT], F32)
        nc.sync.dma_start(out=bk_t, in_=bk_d[:, :])
        bv_t = misc.tile([P, H * QD], F32)
        nc.sync.dma_start(out=bv_t, in_=bv_d[:, :])
        gb_t = misc.tile([P, NIT], F32)
        nc.sync.dma_start(out=gb_t, in_=gb_d[:, :])
        gbh_t = misc.tile([P, NIT], F32)
        nc.sync.dma_start(out=gbh_t, in_=gbh_d[:, :])
        vbh_t = misc.tile([P, NIT], F32)
        nc.sync.dma_start(out=vbh_t, in_=vbh_d[:, :])

        x2_pool = top.enter_context(tc.tile_pool(name="x2_pool", bufs=1))
        X2 = x2_pool.tile([P, NTQ, D], BF)
        asb_pool = top.enter_context(tc.tile_pool(name="asb_pool", bufs=1))
        attn_sb = asb_pool.tile([P, NDT, TQ], F8)
        wop = top.enter_context(tc.tile_pool(name="wo_pool", bufs=1))
        wo_sb = wop.tile([P, NPR, 2, D], F8)
        nc.sync.dma_start(out=wo_sb, in_=wo_d[:, :, :, :])

        # Batched LayerNorm -> transposed (feature-major) output.
        def ln_stats(scope, src, statp, t):
            with nc.named_scope(scope):
                x_t = src(t)
                stats = statp.tile([P, 2, 6], F32, tag="stats",
                                   name=f"st_{scope}_{t}")
                xv = x_t.rearrange("p (c f) -> p c f", f=512)
                for c in range(2):
                    nc.vector.bn_stats(out=stats[:, c, :], in_=xv[:, c, :])
                mv = statp.tile([P, 2], F32, tag="mv", name=f"mv_{scope}_{t}")
                nc.vector.bn_aggr(out=mv, in_=stats)
                return mv

        def ln_rstd(scope, statp, mv, t):
            with nc.named_scope(scope):
                rstd = statp.tile([P, 1], F32, tag="rstd",
                                  name=f"rstd_{scope}_{t}")
                nc.scalar.activation(out=rstd, in_=mv[:, 1:2], func=AF.Sqrt,
                                     bias=eps_t[:, 0:1], scale=1.0)
                nc.vector.reciprocal(out=rstd, in_=rstd)
                return rstd

        def ln_norm_t(scope, src, nrmp, tpp, mv, rstd, dst4, dst_dt, t):
            with nc.named_scope(scope):
                nrm = nrmp.tile([P, D], BF, tag="nrm", name=f"n_{scope}_{t}")
                nc.vector.tensor_scalar(
                    out=nrm, in0=src(t), scalar1=mv[:, 0:1], scalar2=rstd,
                    op0=OP.subtract, op1=OP.mult)
                for half in range(2):
                    tp = tpp.tile([P, 512], BF, tag="fa",
                                  name=f"tp_{scope}_{t}_{half}")
                    for j in range(4):
                        dt = half * 4 + j
                        nc.tensor.transpose(
                            tp[:, j * P:(j + 1) * P],
                            nrm[:, dt * P:(dt + 1) * P], identity)
                    nc.scalar.activation(
                        out=dst4(half, t),
                        in_=tp.rearrange("p (j f) -> p j f", f=P),
                        func=AF.Copy)
                _ = dst_dt  # dst dtype is carried by dst4's target tile

        # ---------------- attention-wide pools ---------------------------
        with tc.tile_pool(name="qkv_sb", bufs=1) as qkvp, \
             tc.tile_pool(name="expp", bufs=3) as expp, \
             tc.tile_pool(name="rvp", bufs=3) as rvp, \
             tc.tile_pool(name="sc_ps", bufs=2, space="PSUM") as scps, \
             tc.tile_pool(name="us_ps", bufs=2, space="PSUM") as usps, \
             tc.tile_pool(name="fil_ps", bufs=2, space="PSUM") as filps:

            Q_all = qkvp.tile([P, NG, 2, TQ], F8)
            K_all = qkvp.tile([P, NG, 2, S], F8)
            V_all = qkvp.tile([P, NG, NT, 4, VW], F8)

            def qkv_mms(g, wpool):
                """Closures emitting group g's QKV projections (fp8 DR)."""
                mms = []
                st = {}

                def alloc():
                    with nc.named_scope(f"qkv{g}"):
                        for nm, wd in (("wq", wq_d), ("wk", wk_d),
                                       ("wv", wv_d)):
                            wt = wpool.tile([P, NPR, 2, 256], F8, tag=nm,
                                            name=f"{nm}{g}")
                            nc.sync.dma_start(
                                out=wt,
                                in_=wd[:, :, :, g * 256:(g + 1) * 256])
                            st[nm] = wt
                        nc.vector.tensor_copy(
                            out=V_all[:, g, :, :, QD],
                            in_=ones64.rearrange("p (k h) -> p k h", h=4)
                            [:, 0:NT, :])
                mms.append(alloc)

                def mk_qk(which, pj, chunk):
                    # whole accumulation chain as one closure so the "fa"
                    # psum slot's lifecycle can't interleave with another
                    # chain's
                    def f():
                        with nc.named_scope(f"qkv{g}"):
                            ps = filps.tile([P, 512], F32, tag="fa",
                                            name=f"{which}ps{g}{pj}{chunk}")
                            for pr in range(NPR):
                                w = st[which][:, pr, :, pj * P:(pj + 1) * P]
                                nc.tensor.matmul(
                                    ps, w,
                                    hT[:, 2 * pr:2 * pr + 2,
                                       chunk * 512:(chunk + 1) * 512],
                                    start=(pr == 0), stop=(pr == NPR - 1),
                                    perf_mode=DRM)
                            b = bq_t if which == "wq" else bk_t
                            dst = Q_all if which == "wq" else K_all
                            dt_g = g * 2 + pj
                            nc.vector.tensor_scalar(
                                out=dst[:, g, pj,
                                        chunk * 512:(chunk + 1) * 512],
                                in0=ps, scalar1=0.125,
                                scalar2=b[:, dt_g:dt_g + 1],
                                op0=OP.mult, op1=OP.add)
                    return f

                def mk_v(kt2):
                    def f():
                        with nc.named_scope(f"qkv{g}"):
                            ps = filps.tile([P, 512], F32, tag="fa",
                                            name=f"vps{g}_{kt2}")
                            for pr in range(NPR):
                                nc.tensor.matmul(
                                    ps[:, 0:256],
                                    hT[:, 2 * pr:2 * pr + 2,
                                       kt2 * P:(kt2 + 1) * P],
                                    st["wv"][:, pr, :, :],
                                    start=(pr == 0), stop=(pr == NPR - 1),
                                    perf_mode=DRM)
                            nc.vector.scalar_tensor_tensor(
                                out=V_all[:, g, kt2, :, 0:QD],
                                in0=ps[:, 0:256].rearrange(
                                    "p (h c) -> p h c", c=QD),
                                scalar=2.0,
                                in1=bv_t.rearrange("p (h c) -> p h c", c=QD)
                                [:, 4 * g:4 * g + 4, :],
                                op0=OP.mult, op1=OP.add)
                    return f

                if g == 0:
                    # split for the LN1-overlapped start: pre = minimum to
                    # start the (g0, pj0) pipeline; named chunks for
                    # deadline-ordered filling by the caller
                    pre = [alloc, mk_qk("wq", 0, 0), mk_qk("wk", 0, 0),
                           mk_v(0), mk_v(1)]
                    parts = {}
                    for c in (1, 2, 3):
                        parts["k0", c] = [mk_qk("wk", 0, c)]
                    for c in range(4):
                        parts["k1", c] = [mk_qk("wk", 1, c)]
                    for kt2 in range(2, NT):
                        parts["v", kt2] = [mk_v(kt2)]
                    for pj in range(2):
                        for qc in range(2):
                            if (pj, qc) != (0, 0):
                                parts["q", pj, qc] = [mk_qk("wq", pj, qc)]
                    return pre, parts
                # ordered so chains needing late LN1 tiles (hT tokens 1024+)
                # sit deep enough in the dosed stream that g=1's fillers
                # (running during g0, concurrent with the LN1 tail) never
                # get emitted before the hT writes they read
                for pj in range(2):
                    for qc in range(2):
                        mms.append(mk_qk("wq", pj, qc))
                for pj in range(2):
                    mms.append(mk_qk("wk", pj, 0))
                    mms.append(mk_qk("wk", pj, 1))
                for kt2 in range(8):
                    mms.append(mk_v(kt2))
                for pj in range(2):
                    mms.append(mk_qk("wk", pj, 2))
                for kt2 in range(8, 12):
                    mms.append(mk_v(kt2))
                for pj in range(2):
                    mms.append(mk_qk("wk", pj, 3))
                for kt2 in range(12, NT):
                    mms.append(mk_v(kt2))
                return mms

            def attn_wave(qc, group_fill, step_fill=None, norm_sink=None):
                """Attention for query chunk qc as ONE software pipeline over
                all 64 (g, pj, kb) steps. group_fill[g] closures are dosed in
                during group g's 16 steps; step_fill[i] closures run entirely
                at step i (deadline-scheduled work). The reciprocal half of
                each softmax normalization is handed to norm_sink instead of
                running inline, so slow DVE ops never block PSUM drains."""
                steps = [(g, pj, kb) for g in range(NG)
                         for pj in range(2) for kb in range(8)]
                step_fill = step_fill or {}
                fi = [0] * NG
                acc = [0.0] * NG

                def fill(g, frac):
                    filler = group_fill[g]
                    acc[g] += frac
                    while acc[g] >= 1.0 and fi[g] < len(filler):
                        filler[fi[g]]()
                        fi[g] += 1
                        acc[g] -= 1.0

                def flush(g):
                    acc[g] += len(group_fill[g])
                    fill(g, 0.0)

                exps = {}
                us = {}

                def emit_scores(step):
                    g, pj, kb = step
                    with nc.named_scope(f"attn{g}w{qc}"):
                        pA = scps.tile([P, 2, 512], F32, tag="s", name="pA")
                        pB = scps.tile([P, 2, 512], F32, tag="s", name="pB")
                        for i, kc in enumerate((2 * kb, 2 * kb + 1)):
                            nc.tensor.matmul(
                                pA[:, i, :],
                                K_all[0:64, g, pj, kc * P:(kc + 1) * P],
                                Q_all[0:64, g, pj,
                                      qc * 512:(qc + 1) * 512],
                                start=True, stop=True,
                                tile_position=(0, 0))
                            nc.tensor.matmul(
                                pB[:, i, :],
                                K_all[64:128, g, pj, kc * P:(kc + 1) * P],
                                Q_all[64:128, g, pj,
                                      qc * 512:(qc + 1) * 512],
                                start=True, stop=True,
                                tile_position=(64, 0))
                        eA = expp.tile([P, 2, 512], F8, tag="eA", name="eA")
                        nc.scalar.activation(out=eA, in_=pA, func=AF.Exp,
                                             bias=nb_t[:, 0:1], scale=0.125)
                        eB = expp.tile([P, 2, 512], F8, tag="eB", name="eB")
                        nc.scalar.activation(out=eB, in_=pB, func=AF.Exp,
                                             bias=nb_t[:, 0:1], scale=0.125)
                        exps[step] = (eA, eB)

                def emit_attnv(step):
                    g, pj, kb = step
                    with nc.named_scope(f"attn{g}w{qc}"):
                        if kb == 0:
                            us[g, pj] = [
                                usps.tile([65, 512], F32, tag="u",
                                          name=f"uA{g}{pj}{qc}"),
                                usps.tile([65, 512], F32, tag="u",
                                          name=f"uB{g}{pj}{qc}")]
                        eA, eB = exps.pop(step)
                        for side, e in ((0, eA), (1, eB)):
                            hh = pj * 2 + side
                            nc.tensor.matmul(
                                us[g, pj][side],
                                V_all[:, g, 2 * kb:2 * kb + 2, hh, 0:65],
                                e, start=(kb == 0), stop=(kb == 7),
                                perf_mode=DRM)

                def emit_norm(g, pj):
                    dt_g = g * 2 + pj
                    for side in (0, 1):
                        u = us[g, pj][side]
                        # evacuate PSUM fast (bf16) so the next chain's
                        # accumulator slot frees without waiting for the
                        # reciprocal chain
                        with nc.named_scope(f"attn{g}w{qc}"):
                            ucp = rvp.tile([65, 512], BF, tag="ucp",
                                           name="ucp", bufs=12)
                            nc.vector.tensor_copy(out=ucp, in_=u)

                        def norm_rest(ucp=ucp, side=side, dt_g=dt_g):
                            with nc.named_scope(f"norm{g}w{qc}"):
                                rv = rvp.tile([1, 512], F32, tag="rv",
                                              name="rv")
                                nc.vector.reciprocal(out=rv[0:1, :],
                                                     in_=ucp[64:65, :])
                                bc = rvp.tile([64, 512], F32, tag="bc",
                                              name="bc")
                                nc.gpsimd.partition_broadcast(bc, rv[0:1, :])
                                nc.vector.tensor_tensor(
                                    out=attn_sb[side * 64:(side + 1) * 64,
                                                dt_g,
                                                qc * 512:(qc + 1) * 512],
                                    in0=ucp[0:64, :], in1=bc, op=OP.mult)
                        norm_sink(g, norm_rest)
                    del us[g, pj]

                LEAD = 1
                for i in range(len(steps) + LEAD):
                    if i < len(steps):
                        g = steps[i][0]
                        if i % 16 == 0 and i > 0:
                            flush(g - 1)  # fillers due before group g starts
                        emit_scores(steps[i])
                        for c in step_fill.get(i, ()):
                            c()
                        fill(g, len(group_fill[g]) / 32.0)
                    if i >= LEAD:
                        step = steps[i - LEAD]
                        emit_attnv(step)
                        if step[2] == 7:
                            emit_norm(step[0], step[1])
                    if i < len(steps):
                        fill(steps[i][0], len(group_fill[steps[i][0]]) / 32.0)
                for g in range(NG):
                    flush(g)

            # ------------- phase A: LN1 + QKV + wave 0 --------------------
            with tc.tile_pool(name="hT_pool", bufs=1) as hT_pool:
                hT = hT_pool.tile([P, NDT, S], F8)

                with tc.tile_pool(name="ln1x", bufs=4) as xp, \
                     tc.tile_pool(name="ln1n", bufs=3) as nrmp, \
                     tc.tile_pool(name="ln1s", bufs=3) as statp, \
                     tc.tile_pool(name="wtl", bufs=2) as wpool:

                    def ln1_tile_cls(t):
                        """LN1 for token tile t as 3 closures (bounded DVE
                        lumps so PSUM-drain ops behind them never starve)."""
                        st = {}

                        def c1():
                            x_t = xp.tile([P, D], F32, tag="x",
                                          name=f"x_ln1_{t}")
                            nc.sync.dma_start(
                                out=x_t, in_=xkv_d[t * P:(t + 1) * P, :])
                            st["x"] = x_t
                            st["mv"] = ln_stats("ln1", lambda _t: x_t,
                                                statp, t)

                        def c2():
                            st["rstd"] = ln_rstd("ln1", statp, st["mv"], t)

                        def c3():
                            ln_norm_t("ln1", lambda _t: st["x"], nrmp, filps,
                                      st["mv"], st["rstd"],
                                      lambda half, _t: hT[:, half * 4:half * 4 + 4,
                                                          _t * P:(_t + 1) * P],
                                      F8, t)
                        return [c1, c2, c3]

                    for t in range(NT):
                        for c in ln1_tile_cls(t):
                            c()
                    pre0, p0 = qkv_mms(0, wpool)
                    for c in pre0 + p0["k0", 1] + p0["k0", 2] + p0["k0", 3]:
                        c()

                    # rest of g0's QKV fed just ahead of its consumers
                    sf0 = {
                        0: p0["v", 2] + p0["v", 3] + p0["v", 4] + p0["v", 5],
                        1: p0["v", 6] + p0["v", 7] + p0["v", 8] + p0["v", 9],
                        2: p0["v", 10] + p0["v", 11] + p0["v", 12] + p0["v", 13],
                        3: p0["v", 14] + p0["v", 15],
                        6: p0["q", 1, 0] + p0["k1", 0],
                        8: p0["k1", 1],
                        10: p0["k1", 2],
                        12: p0["k1", 3],
                        14: p0["q", 0, 1] + p0["q", 1, 1],
                    }
                    w0_norm_tail = []  # wave-0 g3 norms, run at wave-1 start
                    w0_deferred = []   # wave-0 g0-g2 norms, run during g3

                    def w0_sink(g, cl):
                        (w0_deferred if g < 3 else w0_norm_tail).append(cl)

                    attn_wave(0, [qkv_mms(1, wpool), qkv_mms(2, wpool),
                                  qkv_mms(3, wpool), w0_deferred],
                              step_fill=sf0, norm_sink=w0_sink)

            # ------------- phase B: wave 1 + MLP --------------------------
            with tc.tile_pool(name="h2T_pool", bufs=1) as h2T_pool, \
                 tc.tile_pool(name="m_pool", bufs=1) as mp, \
                 tc.tile_pool(name="ln2s", bufs=4) as statp2, \
                 tc.tile_pool(name="ln2n", bufs=3) as nrmp2, \
                 tc.tile_pool(name="opx", bufs=2) as oxp, \
                 tc.tile_pool(name="gvw", bufs=3) as gvwp, \
                 tc.tile_pool(name="gvt", bufs=3) as gvtp, \
                 tc.tile_pool(name="oww", bufs=6) as owwp, \
                 tc.tile_pool(name="owd", bufs=2) as owdp:
                h2T = h2T_pool.tile([P, NDT, TQ], BF)
                m_sb = mp.tile([P, NIT, 512], BF)  # one wave at a time

                def outproj_cls(mt):
                    cls = []
                    st = {}

                    def load():
                        with nc.named_scope("outproj"):
                            st["xq"] = oxp.tile([P, D], F32, tag="xq",
                                                name=f"xq{mt}")
                            nc.sync.dma_start(
                                out=st["xq"],
                                in_=xq_d[mt * P:(mt + 1) * P, :])
                    cls.append(load)

                    def mk(ncx):
                        def f():
                            with nc.named_scope("outproj"):
                                ps = filps.tile([P, 512], F32, tag="fa",
                                                name=f"ops{mt}{ncx}")
                                for pr in range(NPR):
                                    nc.tensor.matmul(
                                        ps,
                                        attn_sb[:, 2 * pr:2 * pr + 2,
                                                mt * P:(mt + 1) * P],
                                        wo_sb[:, pr, :,
                                              ncx * 512:(ncx + 1) * 512],
                                        start=(pr == 0), stop=(pr == NPR - 1),
                                        perf_mode=DRM)
                                nc.vector.scalar_tensor_tensor(
                                    out=X2[:, mt, ncx * 512:(ncx + 1) * 512],
                                    in0=ps, scalar=1.0 / 128.0,
                                    in1=st["xq"][:, ncx * 512:(ncx + 1) * 512],
                                    op0=OP.mult, op1=OP.add)
                        return f
                    for ncx in range(2):
                        cls.append(mk(ncx))
                    return cls

                def ln2_cls(mts):
                    """LN2 for the given query tiles; sqrt batched so the
                    ACT-table swaps away from Exp happen once."""
                    cls = []
                    mvs = {}
                    rstds = {}

                    def mk_stats(t):
                        def f():
                            mvs[t] = ln_stats("ln2", lambda _t: X2[:, _t, :],
                                              statp2, t)
                        return f

                    def rstd_batch():
                        for t in mts:
                            rstds[t] = ln_rstd("ln2", statp2, mvs[t], t)

                    def mk_fin(t):
                        def f():
                            ln_norm_t("ln2", lambda _t: X2[:, _t, :], nrmp2,
                                      filps, mvs[t], rstds[t],
                                      lambda half, _t: h2T[:, half * 4:half * 4 + 4,
                                                           _t * P:(_t + 1) * P],
                                      BF, t)
                        return f
                    for t in mts:
                        cls.append(mk_stats(t))
                    cls.append(rstd_batch)
                    for t in mts:
                        cls.append(mk_fin(t))
                    return cls

                def gv_cls(it, qc2, use_sc_psum=False):
                    cls = []
                    st = {}

                    def load():
                        with nc.named_scope("mlp_gv"):
                            st["g"] = gvwp.tile([P, NDT, P], BF, tag="gsl",
                                                name=f"gsl{it}")
                            nc.sync.dma_start(
                                out=st["g"], in_=gw_d[:, it * P:(it + 1) * P]
                                .rearrange("(kt p) n -> p kt n", p=P))
                            st["v"] = gvwp.tile([P, NDT, P], BF, tag="vsl",
                                                name=f"vsl{it}")
                            nc.sync.dma_start(
                                out=st["v"], in_=vw_d[:, it * P:(it + 1) * P]
                                .rearrange("(kt p) n -> p kt n", p=P))
                    cls.append(load)

                    def alloc_ps():
                        if use_sc_psum:
                            a = scps.tile([P, 2, 512], F32, tag="s",
                                          name=f"gvps{it}")
                            st["gps"], st["vps"] = a[:, 0, :], a[:, 1, :]
                        else:
                            st["gps"] = filps.tile([P, 512], F32, tag="fa",
                                                   name=f"psg{it}")
                            st["vps"] = filps.tile([P, 512], F32, tag="fa",
                                                   name=f"psv{it}")

                    def mk_mm(which, kt):
                        def f():
                            with nc.named_scope("mlp_gv"):
                                if which == "g" and kt == 0:
                                    alloc_ps()
                                nc.tensor.matmul(
                                    st[which + "ps"], st[which][:, kt, :],
                                    h2T[:, kt, qc2 * 512:(qc2 + 1) * 512],
                                    start=(kt == 0), stop=(kt == NDT - 1))
                        return f
                    for kt in range(NDT):
                        cls.append(mk_mm("g", kt))
                    for kt in range(NDT):
                        cls.append(mk_mm("v", kt))

                    def drain():
                        with nc.named_scope("mlp_gv"):
                            g_sb = gvtp.tile([P, 512], BF, tag="g_sb",
                                             name=f"g_sb{it}")
                            nc.vector.tensor_copy(out=g_sb, in_=st["gps"])
                            vh = gvtp.tile([P, 512], BF, tag="vh",
                                           name=f"vh{it}")
                            nc.vector.tensor_scalar_add(
                                out=vh, in0=st["vps"],
                                scalar1=vbh_t[:, it:it + 1])
                            t_t = gvtp.tile([P, 512], BF, tag="t",
                                            name=f"t{it}")
                            nc.scalar.activation(out=t_t, in_=g_sb,
                                                 func=AF.Tanh,
                                                 bias=gbh_t[:, it:it + 1],
                                                 scale=0.5)
                            gvh = gvtp.tile([P, 512], BF, tag="gvh",
                                            name=f"gvh{it}")
                            nc.vector.scalar_tensor_tensor(
                                out=gvh, in0=g_sb,
                                scalar=gb_t[:, it:it + 1], in1=vh,
                                op0=OP.add, op1=OP.mult)
                            nc.vector.scalar_tensor_tensor(
                                out=m_sb[:, it, :], in0=t_t, scalar=1.0,
                                in1=gvh, op0=OP.add, op1=OP.mult)
                    cls.append(drain)
                    return cls

                def ow_pass_cls(mts, qc2, accs_fn):
                    """ow output tiles for the given mts (2 psum chains each)
                    sharing one streamed pass over the 22 weight tiles.
                    accs_fn() -> list of 2*len(mts) accumulator APs."""
                    cls = []
                    st = {}

                    def mk_it(it):
                        def f():
                            with nc.named_scope("mlp_ow"):
                                if it == 0:
                                    st["a"] = accs_fn()
                                owt = owwp.tile([P, D], BF, tag="owt",
                                                name=f"owt{mts[0]}_{it}")
                                nc.sync.dma_start(
                                    out=owt,
                                    in_=ow_d[it * P:(it + 1) * P, :])
                                for j, mt in enumerate(mts):
                                    mloc = mt - qc2 * 4
                                    for ncx in range(2):
                                        nc.tensor.matmul(
                                            st["a"][2 * j + ncx],
                                            m_sb[:, it,
                                                 mloc * P:(mloc + 1) * P],
                                            owt[:, ncx * 512:(ncx + 1) * 512],
                                            start=(it == 0),
                                            stop=(it == NIT - 1))
                        return f
                    for it in range(NIT):
                        cls.append(mk_it(it))

                    def drain():
                        with nc.named_scope("mlp_ow"):
                            for j, mt in enumerate(mts):
                                for ncx in range(2):
                                    ot = owdp.tile([P, 512], F32, tag="ot",
                                                   name=f"ot{mt}{ncx}")
                                    nc.vector.tensor_tensor(
                                        out=ot, in0=st["a"][2 * j + ncx],
                                        in1=X2[:, mt,
                                               ncx * 512:(ncx + 1) * 512],
                                        op=OP.add)
                                    nc.sync.dma_start(
                                        out=out_d[mt * P:(mt + 1) * P,
                                                  ncx * 512:(ncx + 1) * 512],
                                        in_=ot)
                    cls.append(drain)
                    return cls

                def fil_accs():
                    a0 = filps.tile([P, 512], F32, tag="fa", name="owa0")
                    a1 = filps.tile([P, 512], F32, tag="fa", name="owa1")
                    return [a0, a1]

                def quad_accs():
                    a = scps.tile([P, 2, 512], F32, tag="s", name="owa2")
                    b0 = filps.tile([P, 512], F32, tag="fa", name="owb0")
                    b1 = filps.tile([P, 512], F32, tag="fa", name="owb1")
                    return [a[:, 0, :], a[:, 1, :], b0, b1]

                # wave-1 filler: wave-0's g3 norms, then outproj, LN2, MLP
                w1_fill = list(w0_norm_tail)
                for mt in range(4):
                    w1_fill += outproj_cls(mt)
                w1_fill += ln2_cls(range(4))
                for it in range(NIT):
                    w1_fill += gv_cls(it, 0)
                for mt in range(2):
                    w1_fill += ow_pass_cls([mt], 0, fil_accs)

                w1_norms = []
                nseg = (len(w1_fill) + NG - 1) // NG
                attn_wave(1, [w1_fill[i * nseg:(i + 1) * nseg]
                              for i in range(NG)],
                          norm_sink=lambda g, cl: w1_norms.append(cl))

                def sc4_accs():
                    a = scps.tile([P, 2, 512], F32, tag="s", name="owa4")
                    b = scps.tile([P, 2, 512], F32, tag="s", name="owb4")
                    return [a[:, 0, :], a[:, 1, :], b[:, 0, :], b[:, 1, :]]

                # drain: merge the DVE/Scalar-led wave-1 norms+outproj+LN2
                # stream with the PE-led remaining wave-0 ow pass so neither
                # engine idles; psum: ow uses "s" slots, outproj/LN2 "fa".
                da = list(w1_norms)
                for mt in range(4, 8):
                    da += outproj_cls(mt)
                da += ln2_cls(range(4, 8))
                db = ow_pass_cls([2, 3], 0, sc4_accs)
                k = max(len(da), len(db))
                for i in range(k):
                    if i < len(da):
                        da[i]()
                    if i < len(db):
                        db[i]()
                for it in range(NIT):
                    for c in gv_cls(it, 1, use_sc_psum=True):
                        c()
                for mts in ([4, 5], [6, 7]):
                    for c in ow_pass_cls(mts, 1, quad_accs):
                        c()
    return nc


def make_core_inputs(X, src_padding_mask, n1_w, n1_b, n2_w, n2_b,
                     wq, bq, wk, bk, wv, bv, wo, bo,
                     gw, gb, vw, vb, ow, ob):
    """Build the per-core device input dicts from full numpy inputs.
    LayerNorm affines are folded into the consuming projections:
    h = z*w + b  =>  h @ W + c = z @ (diag(w) W) + (b W + c)."""
    X = np.asarray(X, np.float32)
    f = lambda a: np.ascontiguousarray(np.asarray(a, np.float32))
    n1_w, n1_b = f(n1_w), f(n1_b)
    n2_w, n2_b = f(n2_w), f(n2_b)
    wq_f = n1_w[:, None] * f(wq)
    wk_f = n1_w[:, None] * f(wk)
    wv_f = n1_w[:, None] * f(wv)
    bq_f = f(bq) + n1_b @ f(wq)
    bk_f = f(bk) + n1_b @ f(wk)
    bv_f = f(bv) + n1_b @ f(wv)
    gw_f = n2_w[:, None] * f(gw)
    vw_f = n2_w[:, None] * f(vw)
    gb_f = f(gb) + n2_b @ f(gw)
    vb_f = f(vb) + n2_b @ f(vw)

    # pack [D, D] -> [P, NPR, 2, D] fp8 with x8 scale (DoubleRow k-planes)
    def pack8(w):
        w8 = (8.0 * w).reshape(NPR, 2, P, D).transpose(2, 0, 1, 3)
        return np.ascontiguousarray(w8).astype(E4NP)

    col = lambda v: f(v).reshape(NDT, P).T.copy()       # [P, 8] per-partition
    coli = lambda v: np.pad(f(v), (0, INNER_PAD - INNER)).reshape(NIT, P).T.copy()
    shared = {
        "wq_p": pack8(wq_f), "wk_p": pack8(wk_f), "wv_p": pack8(wv_f),
        "wo_p": pack8(f(wo)),
        "bq_t": col(bq_f), "bk_t": col(bk_f),
        "bv_t": np.tile(16.0 * bv_f, (P, 1)).astype(np.float32),
        "gw_p": np.pad(gw_f, ((0, 0), (0, INNER_PAD - INNER))).astype(BFNP),
        "vw_p": np.pad(0.5 * vw_f,
                       ((0, 0), (0, INNER_PAD - INNER))).astype(BFNP),
        "gb_t": coli(gb_f), "gbh_t": coli(0.5 * gb_f),
        "vbh_t": coli(0.5 * vb_f),
        "ow_p": np.pad(f(ow), ((0, INNER_PAD - INNER), (0, 0))).astype(BFNP),
    }
    res_b = (f(bo) + f(ob))[None, :]
    in_maps = []
    for c in range(8):
        b, q0 = c // 2, (c % 2) * TQ
        xroll = np.ascontiguousarray(
            np.concatenate([X[b, q0:], X[b, :q0]], axis=0))
        m = dict(shared)
        m["xkv"] = xroll
        m["xq_res"] = np.ascontiguousarray(xroll[:TQ] + res_b)
        in_maps.append(m)
    return in_maps


_CACHE = {}


def _get_compiled():
    if "nc" not in _CACHE:
        nc = build_nc()
        nc.compile()
        _CACHE["nc"] = nc
    return _CACHE["nc"]


def kernel(**inputs) -> np.ndarray:
    nc = _get_compiled()
    in_maps = make_core_inputs(**inputs)
    res = run_bass_kernel_spmd(nc, in_maps, core_ids=list(range(8)))
    B_full, S_full = 4, 2048
    out = np.empty((B_full, S_full, D), np.float32)
    for c in range(8):
        b, q0 = c // 2, (c % 2) * TQ
        out[b, q0:q0 + TQ, :] = res.results[c]["out"]
    return out
